# revision 25
# baseline (speedup 1.0000x reference)
"""Trainium2 Bass kernel for the non-local attention block (nn_ASM_5196910428634).

8 NeuronCores, data-parallel over batch (1 element per core).  Per core:
  x' = fuse[b] + bias2 as [C=256, HW=4096] fp16 (bias2 = the BN-folded
       W-conv bias; every projection bias is adjusted by -W @ bias2)
  theta = Wt @ x' + bt'              [128, 4096]   (ScalarE copy w/ bias)
  phi   = pool2(Wp @ x') + bp'       [128, 1024]   (2x2-maxpooled STRAIGHT
                                      from PSUM in one DVE tensor_reduce
                                      (axis=XY over the window dims); max
                                      commutes with the per-channel bias, so
                                      the bias lands on the 4x-smaller
                                      pooled map via a short ScalarE ACT)
  g     = pool2(Wg @ x') + bg'       [128, 1024]   (same pooling; bias added
                                      once on the pooled map by DVE)
  fT[k, n] = sum_ci phi[ci, k] theta[ci, n]        (fp16 matmul, k=1024)
  A = exp(fT)  -> bf16               (softmax w/o max-subtraction: |f| << 88,
                                      bf16 carries the fp32 exponent range)
  sums[n]: pairwise tree over A's 8 k-chunks (level 1 on GpSimd, level 2 on
           DVE), then 2 accumulating all-ones matmuls whose output rows ARE
           the partition-reduced broadcast
  yT[c, n] = sum_k gT[k, c] A[k, n]  (bf16 matmul, unnormalized)
  y_norm = yT * (1/sums)             (reciprocal_approx_fast + STT on DVE)
  z_p = WW' @ y_norm                 (BN scale host-folded into WW'; ONE DVE
                                      copy evacuates psW -> fp16 per window)
Host assembles out = concat([lc, z_p + fuse + bias2, gc], axis=1) -- the
residual ride is exact f32 host math during the (required anyway) unshard.

Schedule notes:
- the NEFF wrapper prologue/epilogue (~14us: engine barriers, TENSOR_LOADs,
  per-semaphore resets) is fixed overhead outside the bass program.
- steady state is ScalarE-exp-bound: the 32 exps issue back-to-back at
  ~1.01us each (N=1024/partition), window period ~4.3us with DVE ~94% busy
  (tree level 2, reciprocal, ynorm, z evacuation are all PSUM-bound ops
  that only DVE/ScalarE can touch -- GpSimd cannot read PSUM, and its
  tensor_scalar is ~25 cyc/elem so only plain adds live there).
- DMA: ~110GB/s per engine HWDGE queue, so x rides TWO queues (t0-half +
  phi weights on SP, t1-half + theta/g weights on Scalar) and bpk/wW ride
  GpSimd SWDGE; all 2KB rows.
- 8 dummy 256-col matmuls bank HAM p-state credit through the DMA head
  without queue-blocking the first conv (x lands ~10.5us; a 512-col warmup
  held the PE queue to ~13us); conv pscs rotate over ppf(2 bufs) + ppw so
  the 3 convs + window-0/1
  front-run (f_pair(0,q-1) AND f_pair(1,q-1) per conv chunk) never starve
  the exp stream while PE is conv-busy.
- steady loop per window w: reduce_b(w-1), consume_w(w-2), f(w,2..3)/exps,
  ones(w-1)+y(w-1) with f(w+1,0..1) interleaved INSIDE the y matmul stream
  (slots k=2,5) so the next window's exps never wait on the y/ones block;
  reduce_a(w) (A-tree level 1) on GpSimd; final windows consume at lag 1
  and the last window drains through finale() (y partials start per-exp,
  sequential k-sum tree, 512-wide chain).
- PSUM: psf [128,1024]x2 (4 banks) + pss [128,512] (1) + psy [128,512] (1)
  + psW [128,1024] (2, shared with conv-g pscs) = 8 banks.
"""

import numpy as np

import concourse.bass as bass
import concourse.tile as tile
from concourse import bacc, mybir
from concourse.bass_utils import run_bass_kernel_spmd
from concourse.masks import make_identity

F32 = mybir.dt.float32
BF16 = mybir.dt.bfloat16
FP16 = mybir.dt.float16
AX = mybir.AluOpType
AF = mybir.ActivationFunctionType

B, C, HW = 8, 256, 4096
CI = 128
NK = 1024
N_CORES = 8
BN_EPS = 1e-5

WINS = [(i * 512, 512) for i in range(8)]
NW = len(WINS)

NQ = 4  # conv chunks of 1024 cols each


def build_program():
    nc = bacc.Bacc("TRN2", target_bir_lowering=False, debug=False,
                   num_devices=N_CORES)

    x_d = nc.dram_tensor("x", [C, HW], FP16, kind="ExternalInput").ap()
    wq_d = nc.dram_tensor("wq", [128, 6 * 128], FP16, kind="ExternalInput").ap()
    wW_d = nc.dram_tensor("wW", [128, 2 * 128], BF16, kind="ExternalInput").ap()
    bpk_d = nc.dram_tensor("bpk", [128, 3], F32, kind="ExternalInput").ap()
    z_d = nc.dram_tensor("z", [C, HW], FP16, kind="ExternalOutput").ap()

    with tile.TileContext(nc) as tc:
        with (
            tc.tile_pool(name="const", bufs=1) as consts,
            tc.tile_pool(name="xs", bufs=1) as xs,
            tc.tile_pool(name="big", bufs=1) as big,
            tc.tile_pool(name="stage", bufs=2) as stage,
            tc.tile_pool(name="ppf", bufs=2, space="PSUM") as ppf,
            tc.tile_pool(name="pps", bufs=1, space="PSUM") as pps,
            tc.tile_pool(name="ppy", bufs=1, space="PSUM") as ppy,
            tc.tile_pool(name="ppw", bufs=1, space="PSUM") as ppw,
        ):
            # ones first (gpsimd queue is free earliest) so PE warmup can
            # start as soon as possible
            ones_mat = consts.tile([128, 512], BF16, tag="ones_mat",
                                   name="ones_mat")
            nc.gpsimd.memset(ones_mat, 1.0)
            ident = consts.tile([128, 128], F32, tag="ident", name="ident")
            make_identity(nc, ident)

            # ---------------- loads ----------------
            wq = consts.tile([128, 6, 128], FP16, tag="wq", name="wq")
            wW = consts.tile([128, 2, 128], BF16, tag="wW", name="wW")
            bpk = consts.tile([128, 3], F32, tag="bpk", name="bpk")
            biases = bpk[:, 0:3]  # col 0 = phi, 1 = theta, 2 = g

            x_t = [[xs.tile([128, 1024], FP16, tag=f"x{t}{i}",
                            name=f"x{t}{i}") for i in range(NQ)]
                   for t in range(2)]

            # three parallel DMA streams (~110GB/s per engine queue):
            # phi weights then x t0 chunks on SP, theta/g weights then x t1
            # chunks on Scalar, bias + wW on GpSimd SWDGE
            wqf = wq.rearrange("p j c -> p (j c)")
            nc.sync.dma_start(out=wqf[:, 0:256], in_=wq_d[:, 0:256])
            nc.scalar.dma_start(out=wqf[:, 256:768], in_=wq_d[:, 256:768])
            nc.gpsimd.dma_start(out=bpk, in_=bpk_d)
            for i in range(NQ):
                nc.sync.dma_start(
                    out=x_t[0][i], in_=x_d[0:128, i * 1024:(i + 1) * 1024])
                nc.scalar.dma_start(
                    out=x_t[1][i], in_=x_d[128:256, i * 1024:(i + 1) * 1024])
            wWf = wW.rearrange("p j c -> p (j c)")
            nc.gpsimd.dma_start(out=wWf, in_=wW_d)

            # ---------------- PE p-state warmup ----------------
            pwu = pps.tile([128, 512], F32, tag="pss", name="pwu")
            for _ in range(8):
                nc.tensor.matmul(pwu[:, 0:256], ones_mat[:, 0:128],
                                 ones_mat[:, 0:256], start=True, stop=True)

            # ---------------- projections (+ window-0 front-run) ----------
            theta_r = big.tile([128, HW], FP16, tag="theta", name="theta")
            phi_p = big.tile([128, NK], F32, tag="phi_p", name="phi_p")
            phi_r = big.tile([128, NK], FP16, tag="phi", name="phi")
            g_pool = big.tile([128, NK], F32, tag="gpool", name="gpool")
            gT_r = big.tile([128, 8, 128], BF16, tag="gT", name="gT")

            a_tiles = [None] * NW

            def conv(widx, q, pool):
                # 1x1 conv chunk q of projection widx (0=phi, 1=theta, 2=g)
                # into psum; returns the psum tile (bias NOT added here).
                # g rides the (conv-phase-idle) ppw pool so the psc rotation
                # has 3 physical buffers
                psc = pool.tile([128, 1024], F32,
                                tag="psW" if pool is ppw else "psf",
                                name="psc")
                for t in range(2):
                    for s0 in (0, 512):
                        nc.tensor.matmul(
                            psc[:, s0:s0 + 512],
                            wq[:, 2 * widx + t, :],
                            x_t[t][q][:, s0:s0 + 512],
                            start=(t == 0), stop=(t == 1))
                return psc

            def pool_from_psum(psc, dst, q, dst_dt_bias=None):
                # full 2x2 maxpool of conv chunk q straight out of PSUM in
                # ONE DVE tensor_reduce (innermost XY dims = the window),
                # optional bias on the pooled 256 cols via a small ScalarE
                # activation (GpSimd tensor_scalar is ~25 cyc/elem -- avoid)
                a = psc.rearrange("p (h2 th w2 tw) -> p h2 w2 th tw",
                                  h2=8, th=2, w2=32, tw=2)
                nc.vector.tensor_reduce(
                    dst[:, 256 * q:256 * (q + 1)].rearrange(
                        "p (h w) -> p h w", h=8),
                    a, axis=mybir.AxisListType.XY, op=AX.max)
                if dst_dt_bias is not None:
                    out, bias = dst_dt_bias
                    nc.vector.tensor_scalar_add(
                        out[:, 256 * q:256 * (q + 1)],
                        dst[:, 256 * q:256 * (q + 1)], bias)

            def f_pair(w, k2):
                # psf[k-subchunk j, n] for window w, k-chunks (2k2, 2k2+1),
                # then one exp over both -> A[w] bf16
                base, wd = WINS[w]
                if a_tiles[w] is None:
                    a_tiles[w] = big.tile([128, 8, 512], BF16, tag="A",
                                          name=f"A{w}", bufs=3)
                a_t = a_tiles[w]
                sl = slice(base, base + wd)
                psf = ppf.tile([128, 1024], F32, tag="psf", name="psf")
                for j in range(2):
                    nc.tensor.matmul(
                        psf[:, j * wd:(j + 1) * wd],
                        phi_r[:, (2 * k2 + j) * 128:(2 * k2 + j + 1) * 128],
                        theta_r[:, sl], start=True, stop=True)
                nc.scalar.activation(
                    out=a_t[:, 2 * k2:2 * k2 + 2, 0:wd],
                    in_=psf[:, 0:2 * wd], func=AF.Exp)

            for q in range(NQ):
                psc = conv(0, q, ppf)              # phi
                pool_from_psum(psc, phi_p, q, (phi_r, biases[:, 0:1]))
                psc = conv(2, q, ppw)              # g (bias added later)
                pool_from_psum(psc, g_pool, q)
                psc = conv(1, q, ppf)              # theta
                nc.scalar.activation(out=theta_r[:, q * 1024:(q + 1) * 1024],
                                     in_=psc, func=AF.Identity,
                                     bias=biases[:, 1:2])
                if q > 0:
                    # window-0/1 front-run, one q behind: both windows'
                    # k-chunks (2q-2, 2q-1) depend only on phi chunk q-1
                    # and theta chunks 0/1, so the exp stream saturates
                    # ScalarE while the PE is still conv-busy
                    f_pair(0, q - 1)
                    f_pair(1, q - 1)
            f_pair(0, 3)
            f_pair(1, 3)

            # g bias once on the pooled map (DVE, in place), then gT
            # transposes (interleaved with window-1 production)
            nc.vector.tensor_scalar_add(g_pool, g_pool, biases[:, 2:3])
            for k in range(8):
                ptr = ppy.tile([128, 512], F32, tag="psy", name="ptr")
                nc.tensor.transpose(ptr[:, :128],
                                    g_pool[:, k * 128:(k + 1) * 128], ident)
                nc.vector.tensor_copy(gT_r[:, k, :], ptr[:, :128])

            # ---------------- attention pipeline ----------------
            rb1s = [None] * NW
            rcs = [None] * NW
            y_tiles = [None] * NW

            def reduce_a(w):
                # A-tree level 1 (k-chunks 0..3) on GpSimd
                wd = WINS[w][1]
                a_t = a_tiles[w]
                rb1 = big.tile([128, 2, 512], BF16, tag="rb1", name="rb1",
                               bufs=2)
                nc.gpsimd.tensor_add(rb1[:, :, 0:wd], a_t[:, 0:2, 0:wd],
                                     a_t[:, 2:4, 0:wd])
                rb1s[w] = rb1

            def reduce_b(w):
                # level 1 (k-chunks 4..7) + combine on DVE
                wd = WINS[w][1]
                a_t = a_tiles[w]
                rb2 = big.tile([128, 2, 512], BF16, tag="rb2", name="rb2",
                               bufs=2)
                rc = big.tile([128, 2, 512], BF16, tag="rc", name="rc",
                              bufs=2)
                nc.vector.tensor_add(rb2[:, :, 0:wd], a_t[:, 4:6, 0:wd],
                                     a_t[:, 6:8, 0:wd])
                nc.vector.tensor_add(rc[:, :, 0:wd], rb1s[w][:, :, 0:wd],
                                     rb2[:, :, 0:wd])
                rcs[w] = rc

            def ca_ones(w):
                # every pss row = sum_k A[k, n]
                wd = WINS[w][1]
                pss = pps.tile([128, 512], F32, tag="pss", name="pss")
                for j in range(2):
                    nc.tensor.matmul(pss[:, 0:wd], ones_mat[:, 0:128],
                                     rcs[w][:, j, 0:wd],
                                     start=(j == 0), stop=(j == 1))
                return pss

            def ca_y(w, pss, mid=None):
                wd = WINS[w][1]
                a_t = a_tiles[w]
                psy = ppy.tile([128, 512], F32, tag="psy", name="psy")
                for k in range(8):
                    nc.tensor.matmul(psy[:, 0:wd], gT_r[:, k, :],
                                     a_t[:, k, 0:wd],
                                     start=(k == 0), stop=(k == 7))
                    if mid is not None and k in (2, 5):
                        # next window's f MMs ride inside the y stream so
                        # the exp pipeline never waits on the y/ones block
                        mid[k == 5]()
                rbc = stage.tile([128, 512], F32, tag="rbc", name="rbc")
                nc.vector.reciprocal_approx_fast(out=rbc[:, 0:wd],
                                                 in_=pss[:, 0:wd])
                y_r = stage.tile([128, 512], BF16, tag="yr", name="yr")
                y_tiles[w] = y_r
                nc.vector.scalar_tensor_tensor(out=y_r[:, 0:wd],
                                               in0=psy[:, 0:wd], scalar=1.0,
                                               in1=rbc[:, 0:wd], op0=AX.mult,
                                               op1=AX.mult)

            def consume_a(w, y_first=False):
                if y_first:
                    # final window: y overlaps the DVE reduce tail
                    wd = WINS[w][1]
                    a_t = a_tiles[w]
                    psy = ppy.tile([128, 512], F32, tag="psy", name="psy")
                    for k in range(8):
                        nc.tensor.matmul(psy[:, 0:wd], gT_r[:, k, :],
                                         a_t[:, k, 0:wd],
                                         start=(k == 0), stop=(k == 7))
                    pss = ca_ones(w)
                    rbc = stage.tile([128, 512], F32, tag="rbc", name="rbc")
                    nc.vector.reciprocal_approx_fast(out=rbc[:, 0:wd],
                                                     in_=pss[:, 0:wd])
                    y_r = stage.tile([128, 512], BF16, tag="yr", name="yr")
                    y_tiles[w] = y_r
                    nc.vector.scalar_tensor_tensor(out=y_r[:, 0:wd],
                                                   in0=psy[:, 0:wd],
                                                   scalar=1.0,
                                                   in1=rbc[:, 0:wd],
                                                   op0=AX.mult, op1=AX.mult)
                else:
                    ca_y(w, ca_ones(w))

            def consume_w(w):
                # z_p = WW' @ y_norm only -- the residual +x is added on the
                # host during output assembly (from the exact f32 fuse), so
                # the PSUM evacuation is ONE DVE copy per window
                base, wd = WINS[w]
                psW = ppw.tile([128, 1024], F32, tag="psW", name="psW")
                for o in range(2):
                    nc.tensor.matmul(psW[:, o * 512:o * 512 + wd],
                                     wW[:, o, :], y_tiles[w][:, 0:wd],
                                     start=True, stop=True)
                zs = stage.tile([128, 1024], FP16, tag="zs", name="zs",
                                bufs=3)
                nc.vector.tensor_copy(
                    zs.rearrange("p (o c) -> p o c", o=2)[:, :, 0:wd],
                    psW.rearrange("p (o c) -> p o c", o=2)[:, :, 0:wd])
                for o in range(2):
                    nc.sync.dma_start(
                        out=z_d[o * 128:(o + 1) * 128, base:base + wd],
                        in_=zs[:, o * 512:o * 512 + wd])

            def finale(w):
                # last window (256 wide): y partials start as soon as their
                # exps land, the k-sum tree is sequential (only chunks 6/7
                # gate on the final exp), and the whole drain runs at 256
                base, wd = WINS[w]
                q, off = base // 1024, base % 1024
                a_t = a_tiles[w]
                psy = ppy.tile([128, 512], F32, tag="psy", name="psy")
                for k in range(6):
                    nc.tensor.matmul(psy[:, 0:wd], gT_r[:, k, :],
                                     a_t[:, k, 0:wd],
                                     start=(k == 0), stop=False)
                # sequential tree: s2 = rb1[0]+rb1[1] (rb1 from reduce_a on
                # GpSimd covers chunks 0..3), then fold 4/5, then 6/7
                tre = big.tile([128, 4, 512], BF16, tag="tre", name="tre")
                rb1 = rb1s[w]
                nc.vector.tensor_add(tre[:, 0, 0:wd], rb1[:, 0, 0:wd],
                                     rb1[:, 1, 0:wd])
                nc.vector.tensor_add(tre[:, 1, 0:wd], a_t[:, 4, 0:wd],
                                     a_t[:, 5, 0:wd])
                nc.vector.tensor_add(tre[:, 2, 0:wd], tre[:, 0, 0:wd],
                                     tre[:, 1, 0:wd])
                for k in (6, 7):
                    nc.tensor.matmul(psy[:, 0:wd], gT_r[:, k, :],
                                     a_t[:, k, 0:wd],
                                     start=False, stop=(k == 7))
                nc.vector.tensor_add(tre[:, 3, 0:wd], a_t[:, 6, 0:wd],
                                     a_t[:, 7, 0:wd])
                nc.vector.tensor_add(tre[:, 0, 0:wd], tre[:, 2, 0:wd],
                                     tre[:, 3, 0:wd])
                pss = pps.tile([128, 512], F32, tag="pss", name="pss")
                nc.tensor.matmul(pss[:, 0:wd], ones_mat[:, 0:128],
                                 tre[:, 0, 0:wd], start=True, stop=True)
                rbc = stage.tile([128, 512], F32, tag="rbc", name="rbc")
                nc.vector.reciprocal_approx_fast(out=rbc[:, 0:wd],
                                                 in_=pss[:, 0:wd])
                y_r = stage.tile([128, 512], BF16, tag="yr", name="yr")
                nc.vector.scalar_tensor_tensor(out=y_r[:, 0:wd],
                                               in0=psy[:, 0:wd], scalar=1.0,
                                               in1=rbc[:, 0:wd],
                                               op0=AX.mult, op1=AX.mult)
                psW = ppw.tile([128, 1024], F32, tag="psW", name="psW")
                zs = stage.tile([128, 1024], FP16, tag="zs", name="zs",
                                bufs=3)
                for o in range(2):
                    nc.tensor.matmul(psW[:, o * 512:o * 512 + wd],
                                     wW[:, o, :], y_r[:, 0:wd],
                                     start=True, stop=True)
                nc.vector.tensor_copy(
                    zs.rearrange("p (o c) -> p o c", o=2)[:, :, 0:wd],
                    psW.rearrange("p (o c) -> p o c", o=2)[:, :, 0:wd])
                for o in range(2):
                    nc.sync.dma_start(
                        out=z_d[o * 128:(o + 1) * 128, base:base + wd],
                        in_=zs[:, o * 512:o * 512 + wd])

            def consume_tail(w):
                # final window: the post-exp drain chain (ones/recip/ynorm/
                # W/resid/store) runs in two pipelined 256-col halves so the
                # serial tail is half as deep
                base, wdf = WINS[w]
                hw = wdf // 2
                q, off = base // 1024, base % 1024
                a_t = a_tiles[w]
                pss = pps.tile([128, 512], F32, tag="pss", name="pss")
                psy = ppy.tile([128, 512], F32, tag="psy", name="psy")
                psW = ppw.tile([128, 1024], F32, tag="psW", name="psW")
                rbc = stage.tile([128, 512], F32, tag="rbc", name="rbc")
                y_r = stage.tile([128, 512], BF16, tag="yr", name="yr")
                zs = stage.tile([128, 1024], FP16, tag="zs", name="zs",
                                bufs=3)
                for h in (0, 1):
                    sl = slice(h * hw, h * hw + hw)
                    for j in range(2):
                        nc.tensor.matmul(pss[:, sl], ones_mat[:, 0:128],
                                         rcs[w][:, j, sl],
                                         start=(j == 0), stop=(j == 1))
                    for k in range(8):
                        nc.tensor.matmul(psy[:, sl], gT_r[:, k, :],
                                         a_t[:, k, sl],
                                         start=(k == 0), stop=(k == 7))
                    nc.vector.reciprocal_approx_fast(out=rbc[:, sl],
                                                     in_=pss[:, sl])
                    nc.vector.scalar_tensor_tensor(out=y_r[:, sl],
                                                   in0=psy[:, sl],
                                                   scalar=1.0,
                                                   in1=rbc[:, sl],
                                                   op0=AX.mult, op1=AX.mult)
                    for o in range(2):
                        osl = slice(o * 512 + h * hw, o * 512 + h * hw + hw)
                        nc.tensor.matmul(psW[:, osl], wW[:, o, :],
                                         y_r[:, sl], start=True, stop=True)
                        nc.vector.tensor_add(
                            zs[:, osl], psW[:, osl],
                            x_t[o][q][:, off + h * hw:off + h * hw + hw])
                        nc.sync.dma_start(
                            out=z_d[o * 128:(o + 1) * 128,
                                    base + h * hw:base + h * hw + hw],
                            in_=zs[:, osl])

            # prologue for w0/w1, then the steady loop: iteration w emits
            # f(w,2..3) AND f(w+1,0..1) so the exp stream never starves
            reduce_a(0)
            reduce_b(0)
            consume_a(0)
            reduce_a(1)
            f_pair(2, 0)
            f_pair(2, 1)
            for w in range(2, NW):
                reduce_b(w - 1)
                if w < NW - 1:
                    consume_w(w - 2)
                f_pair(w, 2)
                f_pair(w, 3)
                mid = None
                if w + 1 < NW:
                    mid = (lambda v=w + 1: f_pair(v, 0),
                           lambda v=w + 1: f_pair(v, 1))
                ca_y(w - 1, ca_ones(w - 1), mid=mid)
                reduce_a(w)
                if w >= NW - 2:
                    # final (256-wide) windows consume at lag 1 so the
                    # W/z chains don't pile up after the exp stream ends
                    consume_w(w - 1)
            finale(NW - 1)
    nc.compile()
    return nc


_nc_cache = None


def _get_nc():
    global _nc_cache
    if _nc_cache is None:
        _nc_cache = build_program()
    return _nc_cache


def run(inputs, trace=False, **kw):
    lc = np.asarray(inputs["lc"], dtype=np.float32)
    fuse = np.asarray(inputs["fuse"], dtype=np.float32)
    gc = np.asarray(inputs["gc"], dtype=np.float32)

    inv = np.asarray(inputs["bn_gamma"], np.float32) / np.sqrt(
        np.asarray(inputs["bn_var"], np.float32) + BN_EPS)
    bias2 = ((np.asarray(inputs["W_b"], np.float32)
              - np.asarray(inputs["bn_mean"], np.float32)) * inv
             + np.asarray(inputs["bn_beta"], np.float32))

    import ml_dtypes
    # BN scale folded into the W-conv weights; x' = x + bias2 rides the
    # residual, each conv bias is adjusted by -W @ bias2 (note: folding g's
    # bias into x' via (I+WW'Wg)^-1 is exact but that matrix is
    # ill-conditioned (cond ~2e3) -- the blown-up beta wrecks fp16/bf16
    # precision, so g keeps a small device bias on the pooled map instead)
    wWs = np.asarray(inputs["W_w"], np.float32) * inv[:, None]   # [C, CI]
    beta = bias2

    wq = np.empty((128, 6 * 128), np.float32)
    bpk = np.empty((128, 3), np.float32)
    for i, (wn, bn) in enumerate((("phi_w", "phi_b"), ("theta_w", "theta_b"),
                                  ("g_w", "g_b"))):
        wmat = np.asarray(inputs[wn], np.float32)          # [CI, C]
        wt = wmat.T.reshape(2, 128, 128)
        wq[:, 2 * i * 128:(2 * i + 2) * 128] = \
            wt.transpose(1, 0, 2).reshape(128, 256)
        bpk[:, i] = np.asarray(inputs[bn], np.float32) - wmat @ beta
    wq = wq.astype(np.float16)
    wW = wWs.T.reshape(128, 256).astype(ml_dtypes.bfloat16)
    common = {"wq": wq, "wW": wW, "bpk": bpk}
    in_maps = []
    for b in range(B):
        m = dict(common)
        m["x"] = np.ascontiguousarray(
            (fuse[b].reshape(C, HW) + beta[:, None]).astype(np.float16))
        in_maps.append(m)

    nc = _get_nc()
    res = run_bass_kernel_spmd(nc, in_maps, core_ids=list(range(N_CORES)),
                               trace=trace, **kw)

    out = np.empty((B, 3 * C, 64, 64), dtype=np.float32)
    out[:, :C] = lc
    resid = fuse + bias2[None, :, None, None]
    for b in range(B):
        out[b, C:2 * C] = np.asarray(res.results[b]["z"], np.float32) \
            .reshape(C, 64, 64) + resid[b]
    out[:, 2 * C:] = gc
    return out, res


def kernel(**inputs) -> np.ndarray:
    out, _ = run(inputs, trace=False)
    return out
